# revision 7
# baseline (speedup 1.0000x reference)
"""Trainium2 Bass kernel for MinecraftAwareAttention3D, v2.

Full inputs in, full output out. One attention head per NeuronCore (tensor
parallel over num_heads=8); per-head projection partials summed on host.

v2 design (vs v1 baseline at 162.8us):
  * GroupNorm folded on host: per-channel a,b computed from x on host;
    a folded into qkv weights, b folded into per-head qkv biases (cq/ck)
    which are added during PSUM->SBUF evacuation (free); v-path constant
    folded into the host-side residual. No h/hk materialization at all.
  * bf16 moving operands (x, qf, vt, oh) halve DMA and keep 1 cyc/col on
    the PE; all stationary operands (kf, weights, P) stay float32r so the
    PE self-loads them (a 2-byte stationary operand makes legalization
    emit one InstLdweights per matmul -- 1000+ extra PE-queue entries).
  * PV computed transposed (o^T[q, 33] = P^T . [V;1]^T) as 33-column bf16
    matmuls: 41us -> 10.6us of PE time. Softmax denominator rides along as
    column 32; normalization becomes a per-partition scalar multiply.
  * exp split across three engines: exact Exp on ACT (with -1e9 bias
    killing the padded tail keys), Schraudolph fast-exp on DVE and Pool
    (one tensor_scalar: i32 = x*2^23*log2(e) + magic, bitcast to f32).
  * PE stream software-pipelined: PV trails S by LAG chunks; previous
    q-group's transposes/projection interleave into the next q-group's
    S stream so PE never head-of-line blocks on other engines.
"""

import numpy as np
import ml_dtypes

import concourse.bass as bass
import concourse.tile as tile
from concourse import mybir
from concourse.bass_utils import run_bass_kernel_spmd

F32 = mybir.dt.float32
F32R = mybir.dt.float32r
BF16 = mybir.dt.bfloat16
I16 = mybir.dt.int16
AF = mybir.ActivationFunctionType
ALU = mybir.AluOpType
NPBF16 = ml_dtypes.bfloat16

B, C, D_, H_, W_ = 1, 256, 16, 16, 16
N = D_ * H_ * W_          # 4096 spatial positions
HEADS, HD = 8, 32
GROUPS = 8
GSIZE = C // GROUPS
EPS = 1e-5
NEG = -1e9
NF = HD + 18              # fused contraction depth: 32 qk dims + 18 boost dims
NCORES = 8
LAG = 3                   # PV trails S by this many key-chunks

# Schraudolph fast exp: exp(x) ~= bitcast_bf16(int16(x * SEXP + BEXP))
SEXP = 128.0 / np.log(2.0)
BEXP = 127.0 * 128.0 - 5.6

TRACE = False
LAST_RESULT = {}
_CACHE = {}

# exp is split by QUERY COLUMNS between ACT (exact Exp) and DVE
# (Schraudolph): a query's whole softmax row must use ONE approximation --
# mixing exact and approximate key-chunks within a query breaks the
# numerator/denominator error cancellation (3% element noise stops
# cancelling and shows up as ~3e-2 output error). GPSIMD cannot access
# PSUM, so Pool only does SWDGE DMA dispatch.
CA = 272     # of each 512-query block, queries 0:CA -> ACT, CA:512 -> DVE


def _split_waits(nc, max_waits=1):
    """This walrus build only encodes one sync wait per instruction; hoist
    extra waits onto same-engine NOPs inserted just before the instruction."""
    n = 0
    for f in nc.m.functions:
        for bb in f.blocks:
            new_insts = []
            for inst in bb.instructions:
                si = inst.sync_info
                if si is not None and si.on_wait and len(si.on_wait) > max_waits:
                    waits = list(si.on_wait)
                    si.on_wait = waits[-max_waits:]
                    for i in range(0, len(waits) - max_waits, max_waits):
                        n += 1
                        nop = mybir.InstNoOp(name=f"I-wsplit-{n}", ins=[], outs=[])
                        nop.engine = inst.engine
                        nop.sync_info = mybir.SyncInfo(
                            on_wait=waits[i : i + max_waits], on_update=[]
                        )
                        new_insts.append(nop)
                new_insts.append(inst)
            bb.instructions[:] = new_insts
    return n





def _build(nk_pad):
    nkc = nk_pad // 128
    kchunks = [(s, min(512, nk_pad - s)) for s in range(0, nk_pad, 512)]

    nc = bass.Bass()

    # ---- I/O ----
    x2b = nc.dram_tensor("x2b", [C, N], BF16, kind="ExternalInput")
    xcb = nc.dram_tensor("xcb", [C, nk_pad], BF16, kind="ExternalInput")
    lfeat = nc.dram_tensor("lfeat", [18, N], BF16, kind="ExternalInput")
    rfeat = nc.dram_tensor("rfeat", [18, nk_pad], BF16, kind="ExternalInput")
    wblob = nc.dram_tensor("wblob", [128, 2, 3, HD], BF16, kind="ExternalInput")
    pwt = nc.dram_tensor("pwt", [HD, C], BF16, kind="ExternalInput")
    # miscf: cols 0:nkc = abias (per-key-chunk exp bias), col nkc = cq,
    # col nkc+1 = ck (both on partitions 0:32)
    miscf = nc.dram_tensor("miscf", [128, nkc + 2], F32, kind="ExternalInput")
    # identb: cols 0:128 = 128x128 identity, cols 128:161 rows 0:33 = 33x33
    identb = nc.dram_tensor("identb", [128, 161], BF16, kind="ExternalInput")
    onesb = nc.dram_tensor("onesb", [1, nk_pad], BF16, kind="ExternalInput")
    out = nc.dram_tensor("o", [C, N], BF16, kind="ExternalOutput")

    with tile.TileContext(nc) as tc:
        with (
            tc.tile_pool(name="consts", bufs=1) as cp,
            tc.tile_pool(name="live", bufs=1) as lp,
        ):
            # ---- long-lived tiles ----
            xt = [lp.tile([128, N], BF16, name=f"xt{c}") for c in range(2)]
            xkt = [lp.tile([128, nk_pad], BF16, name=f"xkt{c}") for c in range(2)]
            qf = lp.tile([NF, N], BF16)           # [q*scale ; L]
            kf = lp.tile([NF, nk_pad], BF16)      # [k ; R]
            vv = lp.tile([HD + 1, nk_pad], BF16)  # [v ; ones]
            vt = lp.tile([128, nkc, HD + 2], BF16)  # inner stride 34: 4B-aligned psum copies
            wt = cp.tile([128, 2, 3, HD], BF16)
            pwt_t = cp.tile([HD, C], BF16)
            miscf_t = cp.tile([128, nkc + 2], F32)
            identb_t = cp.tile([128, 161], BF16)

            # ---- input DMA ----
            # wblob via SWDGE (Pool is idle early); everything else on the SP
            # HWDGE queue in qg0 data-arrival order: boost rows and the first
            # x blocks first, then the rest of xcb, then the later x blocks.
            nc.gpsimd.dma_start(out=wt, in_=wblob[:, :, :, :])

            qb = 1024                             # x2b column block
            ks = min(512, nk_pad)                 # first xcb slice
            kh = min(((nkc + 1) // 2) * 128, nk_pad)
            nc.sync.dma_start(out=miscf_t, in_=miscf[:, :])
            for c in range(2):
                nc.sync.dma_start(out=xt[c][:, 0:qb], in_=x2b[c * 128 : (c + 1) * 128, 0:qb])
            for c in range(2):
                nc.sync.dma_start(out=xkt[c][:, 0:ks], in_=xcb[c * 128 : (c + 1) * 128, 0:ks])
            nc.sync.dma_start(out=kf[HD:NF, :], in_=rfeat[:, :])
            nc.sync.dma_start(out=qf[HD:NF, :], in_=lfeat[:, :])
            nc.sync.dma_start(out=vv[HD : HD + 1, :], in_=onesb[:, :])
            nc.sync.dma_start(out=identb_t, in_=identb[:, :])
            for c in range(2):
                if ks < kh:
                    nc.sync.dma_start(
                        out=xkt[c][:, ks:kh], in_=xcb[c * 128 : (c + 1) * 128, ks:kh]
                    )
            for c in range(2):
                if kh < nk_pad:
                    nc.sync.dma_start(
                        out=xkt[c][:, kh:nk_pad], in_=xcb[c * 128 : (c + 1) * 128, kh:nk_pad]
                    )
            for c in range(2):
                nc.sync.dma_start(
                    out=xt[c][:, qb : 2 * qb], in_=x2b[c * 128 : (c + 1) * 128, qb : 2 * qb]
                )
            nc.sync.dma_start(out=pwt_t, in_=pwt[:, :])
            for b0 in range(2 * qb, N, qb):
                for c in range(2):
                    nc.sync.dma_start(
                        out=xt[c][:, b0 : b0 + qb],
                        in_=x2b[c * 128 : (c + 1) * 128, b0 : b0 + qb],
                    )

            cq_ap = miscf_t[0:HD, nkc : nkc + 1]
            ck_ap = miscf_t[0:HD, nkc + 1 : nkc + 2]

            # ====== Phase B: paired q-groups, software-pipelined ======
            # exp runs on ACT + DVE only (GPSIMD cannot touch PSUM). Two
            # q-groups are processed in lockstep: the A-group's exp is exact
            # (ACT), the B-group's is Schraudolph (DVE). A query's whole
            # softmax row gets ONE approximation -- mixing exact and
            # approximate key-chunks within a query breaks the numerator /
            # denominator error cancellation (~3% element noise stops
            # cancelling and shows up as ~3e-2 output error). The padded
            # last key-chunk always runs exact on ACT (needs the -1e9
            # per-partition bias); for B-queries that is 1/nkc of the keys,
            # which perturbs the cancellation negligibly.
            kn = len(kchunks)
            LAG = 4

            with (
                tc.tile_pool(name="ps_st", bufs=4, space="PSUM") as ps_st,
                tc.tile_pool(name="ps_pv", bufs=2, space="PSUM") as ps_pv,
                tc.tile_pool(name="ps_misc", bufs=2, space="PSUM") as ps_misc,
                tc.tile_pool(name="ptp", bufs=10) as ptp,
                tc.tile_pool(name="osb", bufs=10) as osbp,
                tc.tile_pool(name="ohp", bufs=3) as ohp,
                tc.tile_pool(name="otsb", bufs=3) as otsb,
                tc.tile_pool(name="rp", bufs=2) as rp,
            ):
                def emit_k(i):
                    s0, w = kchunks[i]
                    ps = ps_misc.tile([HD, 512], F32, name="kps", tag="m")
                    nc.tensor.matmul(ps[:, 0:w], lhsT=wt[:, 0, 1, :],
                                     rhs=xkt[0][:, s0 : s0 + w], start=True, stop=False)
                    nc.tensor.matmul(ps[:, 0:w], lhsT=wt[:, 1, 1, :],
                                     rhs=xkt[1][:, s0 : s0 + w], start=False, stop=True)
                    nc.vector.tensor_scalar_add(
                        out=kf[0:HD, s0 : s0 + w], in0=ps[:, 0:w], scalar1=ck_ap
                    )

                def emit_v(i):
                    s0, w = kchunks[i]
                    ps = ps_misc.tile([HD, 512], F32, name="vps", tag="m")
                    nc.tensor.matmul(ps[:, 0:w], lhsT=wt[:, 0, 2, :],
                                     rhs=xkt[0][:, s0 : s0 + w], start=True, stop=False)
                    nc.tensor.matmul(ps[:, 0:w], lhsT=wt[:, 1, 2, :],
                                     rhs=xkt[1][:, s0 : s0 + w], start=False, stop=True)
                    nc.scalar.copy(out=vv[0:HD, s0 : s0 + w], in_=ps[:, 0:w])

                def emit_tr(i):
                    # V' transpose for the (up to) 4 key-chunks of v chunk i
                    g0 = kchunks[i][0] // 128
                    gn = min(4, nkc - g0)
                    tps = ps_misc.tile([128, 4, HD + 2], BF16, name="tps", tag="m")
                    for j in range(gn):
                        kc = g0 + j
                        nc.tensor.transpose(
                            tps[:, j, 0 : HD + 1], in_=vv[:, kc * 128 : (kc + 1) * 128],
                            identity=identb_t[0:33, 128:161],
                        )
                    nc.vector.tensor_copy(
                        out=vt[:, g0 : g0 + gn, 0 : HD + 1], in_=tps[:, 0:gn, 0 : HD + 1]
                    )

                def emit_q(i):
                    s0 = i * 512
                    ps = ps_misc.tile([HD, 512], F32, name="qps", tag="m")
                    nc.tensor.matmul(ps, lhsT=wt[:, 0, 0, :],
                                     rhs=xt[0][:, s0 : s0 + 512], start=True, stop=False)
                    nc.tensor.matmul(ps, lhsT=wt[:, 1, 0, :],
                                     rhs=xt[1][:, s0 : s0 + 512], start=False, stop=True)
                    nc.scalar.add(out=qf[0:HD, s0 : s0 + 512], in_=ps, add=cq_ap)

                def emit_pv(pvt, jbase, pt, kc):
                    # pvt is zeroed by memset up front and every PV matmul
                    # accumulates (start=False): interleaved start=True
                    # groups within one PSUM bank wipe each other's partial
                    # sums (the hardware zero region is the whole bank).
                    for j in range(4):
                        nc.tensor.matmul(
                            pvt[:, jbase + j, :],
                            lhsT=pt[:, j * 128 : (j + 1) * 128],
                            rhs=vt[:, kc, 0 : HD + 1],
                            start=False, stop=(kc == nkc - 1),
                            skip_group_check=True,
                        )

                def finish_pair(qga, pvt):
                    """Emit the reciprocal + normalizes now (split over ACT
                    and DVE); return deferred closures for the PE transposes
                    / projection / output of this q-group pair."""
                    r = rp.tile([128, 8], F32, name="r", tag="r")
                    nc.vector.reciprocal(out=r, in_=pvt[:, :, 32])
                    osbs = []
                    for j in range(8):
                        osb = osbp.tile([128, HD + 1], BF16, name="osb", tag="osb")
                        if j % 2 == 0:
                            nc.scalar.activation(
                                out=osb, in_=pvt[:, j, :], func=AF.Copy,
                                bias=0.0, scale=r[:, j : j + 1],
                            )
                        else:
                            nc.vector.tensor_scalar_mul(
                                out=osb, in0=pvt[:, j, :], scalar1=r[:, j : j + 1]
                            )
                        osbs.append(osb)
                    ds = []
                    for half in range(2):
                        qg = qga + half
                        q0 = qg * 512
                        otps = ps_misc.tile([HD + 1, 4, 128], BF16, name="otps", tag="m")
                        oh = ohp.tile([HD, 512], BF16, name="oh", tag="oh")

                        def d_transpose(otps=otps, osbs4=osbs[half * 4 : half * 4 + 4]):
                            for j in range(4):
                                nc.tensor.transpose(
                                    otps[:, j, :], in_=osbs4[j],
                                    identity=identb_t[:, 0:128],
                                )

                        def d_ohevac(oh=oh, otps=otps, half=half):
                            nc.vector.tensor_copy(out=oh, in_=otps[0:HD, :, :])

                        def d_proj(c, qg=qg, q0=q0, oh=oh):
                            def f():
                                pp = ps_misc.tile([128, 512], F32, name="pp", tag="m")
                                nc.tensor.matmul(
                                    pp, lhsT=pwt_t[:, c * 128 : (c + 1) * 128],
                                    rhs=oh, start=True, stop=True,
                                )
                                ot = otsb.tile([128, 512], BF16, name="ot", tag="ot")
                                if c == 0:
                                    nc.scalar.copy(out=ot, in_=pp)
                                else:
                                    nc.vector.tensor_copy(out=ot, in_=pp)
                                if c == 0:
                                    nc.sync.dma_start(
                                        out=out[0:128, q0 : q0 + 512], in_=ot,
                                    )
                                else:
                                    nc.gpsimd.dma_start(
                                        out=out[128:256, q0 : q0 + 512], in_=ot,
                                    )
                            return f

                        ds += [d_transpose, d_ohevac, d_proj(0), d_proj(1)]
                    # order: trA, trB, ohA, ohB, projA0, projA1, projB0, projB1
                    return [ds[0], ds[4], ds[1], ds[5], ds[2], ds[3], ds[6], ds[7]]

                # pair-0 insertion schedule (kc-step units) for the deferred
                # k/v/transpose production. k_i must precede the S step that
                # consumes it (PE wait-queue is head-of-line: an S waiting on
                # a k-evac whose matmul sits behind it would deadlock); tr_i
                # before the first PV that reads vt group i.
                import collections as _cl
                p0_ins = _cl.defaultdict(list)
                p0_ins[0].append(("v", 0))
                p0_ins[2].append(("tr", 0))
                for i in range(1, kn):
                    p0_ins[4 * i - 1].append(("k", i))
                    p0_ins[4 * i + 1].append(("v", i))
                    p0_ins[4 * i + 3].append(("tr", i))
                _EMIT = {"k": emit_k, "v": emit_v, "tr": emit_tr}

                # deferred-closure drain points within the next pair's stream
                DEFER_AT = {2: 0, 3: 1, 4: 2, 5: 3, 6: 4, 8: 5, 10: 6, 12: 7}

                emit_q(0)
                emit_k(0)
                emit_q(1)
                deferred = []
                for qp in range(4):
                    qga = 2 * qp
                    lag = min(LAG, nkc - 1)
                    pvt = ps_pv.tile([128, 8, HD + 1], F32, name="pvt", tag="pv")
                    nc.vector.memset(pvt, 0.0)
                    pts = {}
                    pending = dict(p0_ins) if qp == 0 else {}
                    for kc in range(nkc):
                        for half in range(2):
                            qg = qga + half
                            q0 = qg * 512
                            st = ps_st.tile([128, 512], F32, name="st", tag="st")
                            nc.tensor.matmul(
                                st, lhsT=kf[:, kc * 128 : (kc + 1) * 128],
                                rhs=qf[:, q0 : q0 + 512], start=True, stop=True,
                            )
                            pt = ptp.tile([128, 512], BF16, name="pt", tag="pt")
                            if half == 0 or kc == nkc - 1:
                                nc.scalar.activation(
                                    out=pt, in_=st, func=AF.Exp,
                                    bias=miscf_t[:, kc : kc + 1], scale=1.0,
                                )
                            else:
                                nc.vector.tensor_scalar(
                                    out=pt.bitcast(I16), in0=st,
                                    scalar1=SEXP, scalar2=BEXP,
                                    op0=ALU.mult, op1=ALU.add,
                                )
                            pts[(kc, half)] = pt
                        for kind, i in pending.pop(kc, []):
                            _EMIT[kind](i)
                        if deferred and kc in DEFER_AT:
                            deferred[DEFER_AT[kc]]()
                            if DEFER_AT[kc] == 7:
                                deferred = []
                        if kc == 14 and qp < 3:
                            emit_q(qga + 2)
                        if kc == 17 and qp < 3:
                            emit_q(qga + 3)
                        if kc >= lag:
                            for half in range(2):
                                emit_pv(pvt, half * 4, pts.pop((kc - lag, half)), kc - lag)
                    for kc2 in sorted(pending):
                        for kind, i in pending[kc2]:
                            _EMIT[kind](i)
                    # small-nkc safety: fire any deferred closures whose kc
                    # drain point never occurred in this pair's stream
                    for di in sorted(v for k, v in DEFER_AT.items() if k >= nkc):
                        if deferred:
                            deferred[di]()
                    deferred = []
                    for kc in range(nkc - lag, nkc):
                        for half in range(2):
                            emit_pv(pvt, half * 4, pts.pop((kc, half)), kc)
                    deferred = finish_pair(qga, pvt)
                    if qp == 3:
                        for d in deferred:
                            d()
                        deferred = []

    _split_waits(nc)
    return nc


def _numpy_reference(x, block_types, gn_w, gn_b, qkv_w, qkv_b, proj_w, proj_b,
                     is_air, is_wood, is_leaves):
    """Pure-numpy fallback (degenerate case: no non-air keys)."""
    xf = x.reshape(B, C, N).astype(np.float64)
    xs = xf.reshape(B, GROUPS, GSIZE * N)
    mu = xs.mean(axis=2, keepdims=True)
    var = xs.var(axis=2, keepdims=True)
    h = ((xs - mu) / np.sqrt(var + EPS)).reshape(B, C, N)
    h = h * gn_w[None, :, None] + gn_b[None, :, None]
    qkv = np.einsum("oc,bcn->bon", qkv_w.astype(np.float64), h) + qkv_b[None, :, None]
    qkv = qkv.reshape(B, 3, HEADS, HD, N)
    q, k, v = qkv[:, 0], qkv[:, 1], qkv[:, 2]
    attn = np.einsum("bhdn,bhdm->bhnm", q, k) * (HD ** -0.5)
    bf = block_types.reshape(B, N)
    air = is_air[bf]; wood = is_wood[bf]; leaves = is_leaves[bf]
    attn = np.where(air[:, None, None, :] > 0, NEG, attn)
    wo = wood[:, :, None] * wood[:, None, :]
    lo = leaves[:, :, None] * leaves[:, None, :]
    mb = np.clip((wo + lo) * 2.0, 0.0, 10.0)
    pos = np.arange(N); ypos = (pos // W_) % H_
    vm = (np.abs(ypos[None, :] - ypos[:, None]) <= 2).astype(np.float64)
    vb = np.clip(wo * vm[None] * 1.5, 0.0, 10.0)
    attn = attn + (mb + vb)[:, None]
    attn = attn - attn.max(axis=-1, keepdims=True)
    e = np.exp(attn); p = e / e.sum(axis=-1, keepdims=True)
    o = np.einsum("bhnm,bhdm->bhdn", p, v).reshape(B, C, N)
    o = np.einsum("oc,bcn->bon", proj_w.astype(np.float64), o) + proj_b[None, :, None]
    return (xf + o).reshape(x.shape).astype(np.float32)


def kernel(x, block_types, gn_w, gn_b, qkv_w, qkv_b, proj_w, proj_b,
           is_air, is_wood, is_leaves):
    x = np.ascontiguousarray(np.asarray(x, dtype=np.float32))
    gn_w = np.asarray(gn_w, np.float32); gn_b = np.asarray(gn_b, np.float32)
    qkv_w = np.asarray(qkv_w, np.float32); qkv_b = np.asarray(qkv_b, np.float32)
    proj_w = np.asarray(proj_w, np.float32); proj_b = np.asarray(proj_b, np.float32)
    is_air = np.asarray(is_air, np.float32)
    is_wood = np.asarray(is_wood, np.float32)
    is_leaves = np.asarray(is_leaves, np.float32)
    bt = np.asarray(block_types).reshape(N).astype(np.int64)

    x2 = x.reshape(C, N)
    air = is_air[bt]; wood = is_wood[bt]; leaves = is_leaves[bt]
    idx = np.nonzero(air <= 0.0)[0]
    nk = len(idx)
    if nk == 0:
        return _numpy_reference(x, block_types, gn_w, gn_b, qkv_w, qkv_b,
                                proj_w, proj_b, is_air, is_wood, is_leaves)

    nk_pad = ((nk + 127) // 128) * 128
    nkc = nk_pad // 128
    idx_pad = np.concatenate([idx, np.full(nk_pad - nk, idx[0], np.int64)])

    # --- host-side prep: GroupNorm fold + boost features (all O(C*N)) ---
    xs = x2.reshape(GROUPS, GSIZE * N)
    mu = xs.mean(axis=1); var = xs.var(axis=1)
    a_ch = gn_w * np.repeat(1.0 / np.sqrt(var + EPS), GSIZE)
    b_ch = gn_b - np.repeat(mu, GSIZE) * a_ch

    ypos = ((np.arange(N) // W_) % H_).astype(np.int64)
    oneh = np.zeros((N, 16), np.float32); oneh[np.arange(N), ypos] = 1.0
    m16 = (np.abs(np.arange(16)[:, None] - np.arange(16)[None, :]) <= 2).astype(np.float32)
    lfeat = np.ascontiguousarray(np.concatenate(
        [(2.0 * wood)[None], (2.0 * leaves)[None], 1.5 * wood[None] * oneh.T]
    ).astype(NPBF16))                                      # [18, N]
    wood_k = wood[idx_pad]; leaves_k = leaves[idx_pad]
    mk = m16 @ oneh[idx_pad].T
    rfeat = np.ascontiguousarray(np.concatenate(
        [wood_k[None], leaves_k[None], wood_k[None] * mk]
    ).astype(NPBF16))                                      # [18, nk_pad]

    abias = np.zeros(nk_pad, np.float32); abias[nk:] = NEG
    abias = np.ascontiguousarray(abias.reshape(nkc, 128).T)  # [128, nkc]

    identb = np.zeros((128, 161), np.float32)
    identb[:, 0:128] = np.eye(128, dtype=np.float32)
    identb[0:33, 128:161] = np.eye(33, dtype=np.float32)
    identb = identb.astype(NPBF16)

    x2b = np.ascontiguousarray(x2.astype(NPBF16))
    xcb = np.ascontiguousarray(x2[:, idx_pad].astype(NPBF16))

    scale = HD ** -0.5
    shared = {
        "x2b": x2b, "xcb": xcb, "lfeat": lfeat, "rfeat": rfeat, "identb": identb,
        "onesb": np.ones((1, nk_pad), NPBF16),
    }
    in_maps = []
    for hd_i in range(NCORES):
        r0 = hd_i * HD
        wq = qkv_w[r0 : r0 + HD] * scale
        wk = qkv_w[C + r0 : C + r0 + HD]
        wv = qkv_w[2 * C + r0 : 2 * C + r0 + HD]
        cq = wq @ b_ch + qkv_b[r0 : r0 + HD] * scale
        ck = wk @ b_ch + qkv_b[C + r0 : C + r0 + HD]
        wblob = np.zeros((128, 2, 3, HD), np.float32)
        for h in range(2):
            sl = slice(h * 128, (h + 1) * 128)
            wblob[:, h, 0, :] = (wq * a_ch[None, :])[:, sl].T
            wblob[:, h, 1, :] = (wk * a_ch[None, :])[:, sl].T
            wblob[:, h, 2, :] = (wv * a_ch[None, :])[:, sl].T
        miscf = np.zeros((128, nkc + 2), np.float32)
        miscf[:, 0:nkc] = abias
        miscf[0:HD, nkc] = cq
        miscf[0:HD, nkc + 1] = ck
        m = dict(shared)
        m["wblob"] = wblob.astype(NPBF16)
        m["miscf"] = miscf
        m["pwt"] = np.ascontiguousarray(proj_w[:, r0 : r0 + HD].T).astype(NPBF16)
        in_maps.append(m)

    if nk_pad not in _CACHE:
        _CACHE[nk_pad] = _build(nk_pad)
    nc = _CACHE[nk_pad]

    use_trace = TRACE
    if use_trace:
        import importlib.util
        if importlib.util.find_spec("antenv.axon_hooks") is None:
            use_trace = False
    res = run_bass_kernel_spmd(nc, in_maps, core_ids=list(range(NCORES)), trace=use_trace)
    LAST_RESULT["res"] = res

    acc = np.zeros((C, N), np.float32)
    for i in range(NCORES):
        acc += np.asarray(res.results[i]["o"], np.float32)
    # host-folded constants: v-path bias (Wv.b_gn + bv) through projection
    host_c = proj_w @ (qkv_w[2 * C : 3 * C] @ b_ch + qkv_b[2 * C : 3 * C]) + proj_b
    y = x2 + acc + host_c[:, None]
    return y.reshape(B, C, D_, H_, W_).astype(np.float32)


# revision 8
# speedup vs baseline: 1.0035x; 1.0035x over previous
"""Trainium2 Bass kernel for MinecraftAwareAttention3D, v2.

Full inputs in, full output out. One attention head per NeuronCore (tensor
parallel over num_heads=8); per-head projection partials summed on host.

v2 design (vs v1 baseline at 162.8us):
  * GroupNorm folded on host: per-channel a,b computed from x on host;
    a folded into qkv weights, b folded into per-head qkv biases (cq/ck)
    which are added during PSUM->SBUF evacuation (free); v-path constant
    folded into the host-side residual. No h/hk materialization at all.
  * bf16 moving operands (x, qf, vt, oh) halve DMA and keep 1 cyc/col on
    the PE; all stationary operands (kf, weights, P) stay float32r so the
    PE self-loads them (a 2-byte stationary operand makes legalization
    emit one InstLdweights per matmul -- 1000+ extra PE-queue entries).
  * PV computed transposed (o^T[q, 33] = P^T . [V;1]^T) as 33-column bf16
    matmuls: 41us -> 10.6us of PE time. Softmax denominator rides along as
    column 32; normalization becomes a per-partition scalar multiply.
  * exp split across three engines: exact Exp on ACT (with -1e9 bias
    killing the padded tail keys), Schraudolph fast-exp on DVE and Pool
    (one tensor_scalar: i32 = x*2^23*log2(e) + magic, bitcast to f32).
  * PE stream software-pipelined: PV trails S by LAG chunks; previous
    q-group's transposes/projection interleave into the next q-group's
    S stream so PE never head-of-line blocks on other engines.
"""

import numpy as np
import ml_dtypes

import concourse.bass as bass
import concourse.tile as tile
from concourse import mybir
from concourse.bass_utils import run_bass_kernel_spmd

F32 = mybir.dt.float32
F32R = mybir.dt.float32r
BF16 = mybir.dt.bfloat16
I16 = mybir.dt.int16
AF = mybir.ActivationFunctionType
ALU = mybir.AluOpType
NPBF16 = ml_dtypes.bfloat16

B, C, D_, H_, W_ = 1, 256, 16, 16, 16
N = D_ * H_ * W_          # 4096 spatial positions
HEADS, HD = 8, 32
GROUPS = 8
GSIZE = C // GROUPS
EPS = 1e-5
NEG = -1e9
NF = HD + 18              # fused contraction depth: 32 qk dims + 18 boost dims
NCORES = 8
LAG = 3                   # PV trails S by this many key-chunks

# Schraudolph fast exp: exp(x) ~= bitcast_bf16(int16(x * SEXP + BEXP))
SEXP = 128.0 / np.log(2.0)
BEXP = 127.0 * 128.0 - 5.6

TRACE = False
LAST_RESULT = {}
_CACHE = {}

# exp is split by QUERY COLUMNS between ACT (exact Exp) and DVE
# (Schraudolph): a query's whole softmax row must use ONE approximation --
# mixing exact and approximate key-chunks within a query breaks the
# numerator/denominator error cancellation (3% element noise stops
# cancelling and shows up as ~3e-2 output error). GPSIMD cannot access
# PSUM, so Pool only does SWDGE DMA dispatch.
CA = 272     # of each 512-query block, queries 0:CA -> ACT, CA:512 -> DVE


def _split_waits(nc, max_waits=1):
    """This walrus build only encodes one sync wait per instruction; hoist
    extra waits onto same-engine NOPs inserted just before the instruction."""
    n = 0
    for f in nc.m.functions:
        for bb in f.blocks:
            new_insts = []
            for inst in bb.instructions:
                si = inst.sync_info
                if si is not None and si.on_wait and len(si.on_wait) > max_waits:
                    waits = list(si.on_wait)
                    si.on_wait = waits[-max_waits:]
                    for i in range(0, len(waits) - max_waits, max_waits):
                        n += 1
                        nop = mybir.InstNoOp(name=f"I-wsplit-{n}", ins=[], outs=[])
                        nop.engine = inst.engine
                        nop.sync_info = mybir.SyncInfo(
                            on_wait=waits[i : i + max_waits], on_update=[]
                        )
                        new_insts.append(nop)
                new_insts.append(inst)
            bb.instructions[:] = new_insts
    return n





def _build(nk_pad):
    nkc = nk_pad // 128
    kchunks = [(s, min(512, nk_pad - s)) for s in range(0, nk_pad, 512)]

    nc = bass.Bass()

    # ---- I/O ----
    x2b = nc.dram_tensor("x2b", [C, N], BF16, kind="ExternalInput")
    xcb = nc.dram_tensor("xcb", [C, nk_pad], BF16, kind="ExternalInput")
    lfeat = nc.dram_tensor("lfeat", [18, N], BF16, kind="ExternalInput")
    rfeat = nc.dram_tensor("rfeat", [18, nk_pad], BF16, kind="ExternalInput")
    wblob = nc.dram_tensor("wblob", [128, 2, 3, HD], BF16, kind="ExternalInput")
    pwt = nc.dram_tensor("pwt", [HD, C], BF16, kind="ExternalInput")
    # miscf: cols 0:nkc = abias (per-key-chunk exp bias), col nkc = cq,
    # col nkc+1 = ck (both on partitions 0:32)
    miscf = nc.dram_tensor("miscf", [128, nkc + 2], F32, kind="ExternalInput")
    # identb: cols 0:128 = 128x128 identity, cols 128:161 rows 0:33 = 33x33
    identb = nc.dram_tensor("identb", [128, 161], BF16, kind="ExternalInput")
    onesb = nc.dram_tensor("onesb", [1, nk_pad], BF16, kind="ExternalInput")
    out = nc.dram_tensor("o", [C, N], BF16, kind="ExternalOutput")

    with tile.TileContext(nc) as tc:
        with (
            tc.tile_pool(name="consts", bufs=1) as cp,
            tc.tile_pool(name="live", bufs=1) as lp,
        ):
            # ---- long-lived tiles ----
            xt = [lp.tile([128, N], BF16, name=f"xt{c}") for c in range(2)]
            xkt = [lp.tile([128, nk_pad], BF16, name=f"xkt{c}") for c in range(2)]
            qf = lp.tile([NF, N], BF16)           # [q*scale ; L]
            kf = lp.tile([NF, nk_pad], BF16)      # [k ; R]
            vv = lp.tile([HD + 1, nk_pad], BF16)  # [v ; ones]
            vt = lp.tile([128, nkc, HD + 2], BF16)  # inner stride 34: 4B-aligned psum copies
            wt = cp.tile([128, 2, 3, HD], BF16)
            pwt_t = cp.tile([HD, C], BF16)
            miscf_t = cp.tile([128, nkc + 2], F32)
            identb_t = cp.tile([128, 161], BF16)

            # ---- input DMA ----
            # wblob via SWDGE (Pool is idle early); everything else on the SP
            # HWDGE queue in qg0 data-arrival order: boost rows and the first
            # x blocks first, then the rest of xcb, then the later x blocks.
            nc.gpsimd.dma_start(out=wt, in_=wblob[:, :, :, :])

            qb = 1024                             # x2b column block
            ks = min(512, nk_pad)                 # first xcb slice
            kh = min(((nkc + 1) // 2) * 128, nk_pad)
            nc.sync.dma_start(out=miscf_t, in_=miscf[:, :])
            for c in range(2):
                nc.sync.dma_start(out=xt[c][:, 0:qb], in_=x2b[c * 128 : (c + 1) * 128, 0:qb])
            for c in range(2):
                nc.sync.dma_start(out=xkt[c][:, 0:ks], in_=xcb[c * 128 : (c + 1) * 128, 0:ks])
            nc.sync.dma_start(out=kf[HD:NF, :], in_=rfeat[:, :])
            nc.sync.dma_start(out=qf[HD:NF, :], in_=lfeat[:, :])
            nc.sync.dma_start(out=vv[HD : HD + 1, :], in_=onesb[:, :])
            nc.sync.dma_start(out=identb_t, in_=identb[:, :])
            for c in range(2):
                if ks < kh:
                    nc.sync.dma_start(
                        out=xkt[c][:, ks:kh], in_=xcb[c * 128 : (c + 1) * 128, ks:kh]
                    )
            for c in range(2):
                if kh < nk_pad:
                    nc.sync.dma_start(
                        out=xkt[c][:, kh:nk_pad], in_=xcb[c * 128 : (c + 1) * 128, kh:nk_pad]
                    )
            for c in range(2):
                nc.sync.dma_start(
                    out=xt[c][:, qb : 2 * qb], in_=x2b[c * 128 : (c + 1) * 128, qb : 2 * qb]
                )
            nc.sync.dma_start(out=pwt_t, in_=pwt[:, :])
            for b0 in range(2 * qb, N, qb):
                for c in range(2):
                    nc.sync.dma_start(
                        out=xt[c][:, b0 : b0 + qb],
                        in_=x2b[c * 128 : (c + 1) * 128, b0 : b0 + qb],
                    )

            cq_ap = miscf_t[0:HD, nkc : nkc + 1]
            ck_ap = miscf_t[0:HD, nkc + 1 : nkc + 2]

            # ====== Phase B: paired q-groups, software-pipelined ======
            # exp runs on ACT + DVE only (GPSIMD cannot touch PSUM). Two
            # q-groups are processed in lockstep: the A-group's exp is exact
            # (ACT), the B-group's is Schraudolph (DVE). A query's whole
            # softmax row gets ONE approximation -- mixing exact and
            # approximate key-chunks within a query breaks the numerator /
            # denominator error cancellation (~3% element noise stops
            # cancelling and shows up as ~3e-2 output error). The padded
            # last key-chunk always runs exact on ACT (needs the -1e9
            # per-partition bias); for B-queries that is 1/nkc of the keys,
            # which perturbs the cancellation negligibly.
            kn = len(kchunks)
            LAG = 5

            with (
                tc.tile_pool(name="ps_st", bufs=4, space="PSUM") as ps_st,
                tc.tile_pool(name="ps_pv", bufs=2, space="PSUM") as ps_pv,
                tc.tile_pool(name="ps_misc", bufs=2, space="PSUM") as ps_misc,
                tc.tile_pool(name="ptp", bufs=12) as ptp,
                tc.tile_pool(name="osb", bufs=10) as osbp,
                tc.tile_pool(name="ohp", bufs=3) as ohp,
                tc.tile_pool(name="otsb", bufs=3) as otsb,
                tc.tile_pool(name="rp", bufs=2) as rp,
            ):
                def emit_k(i):
                    s0, w = kchunks[i]
                    ps = ps_misc.tile([HD, 512], F32, name="kps", tag="m")
                    nc.tensor.matmul(ps[:, 0:w], lhsT=wt[:, 0, 1, :],
                                     rhs=xkt[0][:, s0 : s0 + w], start=True, stop=False)
                    nc.tensor.matmul(ps[:, 0:w], lhsT=wt[:, 1, 1, :],
                                     rhs=xkt[1][:, s0 : s0 + w], start=False, stop=True)
                    nc.vector.tensor_scalar_add(
                        out=kf[0:HD, s0 : s0 + w], in0=ps[:, 0:w], scalar1=ck_ap
                    )

                def emit_v(i):
                    s0, w = kchunks[i]
                    ps = ps_misc.tile([HD, 512], F32, name="vps", tag="m")
                    nc.tensor.matmul(ps[:, 0:w], lhsT=wt[:, 0, 2, :],
                                     rhs=xkt[0][:, s0 : s0 + w], start=True, stop=False)
                    nc.tensor.matmul(ps[:, 0:w], lhsT=wt[:, 1, 2, :],
                                     rhs=xkt[1][:, s0 : s0 + w], start=False, stop=True)
                    nc.scalar.copy(out=vv[0:HD, s0 : s0 + w], in_=ps[:, 0:w])

                def emit_tr(i):
                    # V' transpose for the (up to) 4 key-chunks of v chunk i
                    g0 = kchunks[i][0] // 128
                    gn = min(4, nkc - g0)
                    tps = ps_misc.tile([128, 4, HD + 2], BF16, name="tps", tag="m")
                    for j in range(gn):
                        kc = g0 + j
                        nc.tensor.transpose(
                            tps[:, j, 0 : HD + 1], in_=vv[:, kc * 128 : (kc + 1) * 128],
                            identity=identb_t[0:33, 128:161],
                        )
                    nc.vector.tensor_copy(
                        out=vt[:, g0 : g0 + gn, 0 : HD + 1], in_=tps[:, 0:gn, 0 : HD + 1]
                    )

                def emit_q(i):
                    s0 = i * 512
                    ps = ps_misc.tile([HD, 512], F32, name="qps", tag="m")
                    nc.tensor.matmul(ps, lhsT=wt[:, 0, 0, :],
                                     rhs=xt[0][:, s0 : s0 + 512], start=True, stop=False)
                    nc.tensor.matmul(ps, lhsT=wt[:, 1, 0, :],
                                     rhs=xt[1][:, s0 : s0 + 512], start=False, stop=True)
                    nc.scalar.add(out=qf[0:HD, s0 : s0 + 512], in_=ps, add=cq_ap)

                def emit_pv(pvt, jbase, pt, kc):
                    # pvt is zeroed by memset up front and every PV matmul
                    # accumulates (start=False): interleaved start=True
                    # groups within one PSUM bank wipe each other's partial
                    # sums (the hardware zero region is the whole bank).
                    for j in range(4):
                        nc.tensor.matmul(
                            pvt[:, jbase + j, :],
                            lhsT=pt[:, j * 128 : (j + 1) * 128],
                            rhs=vt[:, kc, 0 : HD + 1],
                            start=False, stop=(kc == nkc - 1),
                            skip_group_check=True,
                        )

                def finish_pair(qga, pvt):
                    """Emit the reciprocal + normalizes now (split over ACT
                    and DVE); return deferred closures for the PE transposes
                    / projection / output of this q-group pair."""
                    r = rp.tile([128, 8], F32, name="r", tag="r")
                    nc.vector.reciprocal(out=r, in_=pvt[:, :, 32])
                    osbs = []
                    for j in range(8):
                        osb = osbp.tile([128, HD + 1], BF16, name="osb", tag="osb")
                        if j % 2 == 0:
                            nc.scalar.activation(
                                out=osb, in_=pvt[:, j, :], func=AF.Copy,
                                bias=0.0, scale=r[:, j : j + 1],
                            )
                        else:
                            nc.vector.tensor_scalar_mul(
                                out=osb, in0=pvt[:, j, :], scalar1=r[:, j : j + 1]
                            )
                        osbs.append(osb)
                    ds = []
                    for half in range(2):
                        qg = qga + half
                        q0 = qg * 512
                        otps = ps_misc.tile([HD + 1, 4, 128], BF16, name="otps", tag="m")
                        oh = ohp.tile([HD, 512], BF16, name="oh", tag="oh")

                        def d_transpose(otps=otps, osbs4=osbs[half * 4 : half * 4 + 4]):
                            for j in range(4):
                                nc.tensor.transpose(
                                    otps[:, j, :], in_=osbs4[j],
                                    identity=identb_t[:, 0:128],
                                )

                        def d_ohevac(oh=oh, otps=otps, half=half):
                            nc.vector.tensor_copy(out=oh, in_=otps[0:HD, :, :])

                        def d_proj(c, qg=qg, q0=q0, oh=oh):
                            def f():
                                pp = ps_misc.tile([128, 512], F32, name="pp", tag="m")
                                nc.tensor.matmul(
                                    pp, lhsT=pwt_t[:, c * 128 : (c + 1) * 128],
                                    rhs=oh, start=True, stop=True,
                                )
                                ot = otsb.tile([128, 512], BF16, name="ot", tag="ot")
                                if c == 0:
                                    nc.scalar.copy(out=ot, in_=pp)
                                else:
                                    nc.vector.tensor_copy(out=ot, in_=pp)
                                if c == 0:
                                    nc.sync.dma_start(
                                        out=out[0:128, q0 : q0 + 512], in_=ot,
                                    )
                                else:
                                    nc.gpsimd.dma_start(
                                        out=out[128:256, q0 : q0 + 512], in_=ot,
                                    )
                            return f

                        ds += [d_transpose, d_ohevac, d_proj(0), d_proj(1)]
                    # order: trA, trB, ohA, ohB, projA0, projA1, projB0, projB1
                    return [ds[0], ds[4], ds[1], ds[5], ds[2], ds[3], ds[6], ds[7]]

                # pair-0 insertion schedule (kc-step units) for the deferred
                # k/v/transpose production. k_i must precede the S step that
                # consumes it (PE wait-queue is head-of-line: an S waiting on
                # a k-evac whose matmul sits behind it would deadlock); tr_i
                # before the first PV that reads vt group i.
                import collections as _cl
                p0_ins = _cl.defaultdict(list)
                p0_ins[0].append(("v", 0))
                p0_ins[2].append(("tr", 0))
                for i in range(1, kn):
                    p0_ins[4 * i - 1].append(("k", i))
                    p0_ins[4 * i + 1].append(("v", i))
                    p0_ins[4 * i + 3].append(("tr", i))
                _EMIT = {"k": emit_k, "v": emit_v, "tr": emit_tr}

                # deferred-closure drain points within the next pair's stream
                DEFER_AT = {2: 0, 3: 1, 4: 2, 5: 3, 6: 4, 8: 5, 10: 6, 12: 7}

                emit_q(0)
                emit_k(0)
                emit_q(1)
                deferred = []
                for qp in range(4):
                    qga = 2 * qp
                    lag = min(LAG, nkc - 1)
                    pvt = ps_pv.tile([128, 8, HD + 1], F32, name="pvt", tag="pv")
                    nc.vector.memset(pvt, 0.0)
                    pts = {}
                    pending = dict(p0_ins) if qp == 0 else {}
                    for kc in range(nkc):
                        for half in range(2):
                            qg = qga + half
                            q0 = qg * 512
                            st = ps_st.tile([128, 512], F32, name="st", tag="st")
                            nc.tensor.matmul(
                                st, lhsT=kf[:, kc * 128 : (kc + 1) * 128],
                                rhs=qf[:, q0 : q0 + 512], start=True, stop=True,
                            )
                            pt = ptp.tile([128, 512], BF16, name="pt", tag="pt")
                            if half == 0 or kc == nkc - 1:
                                nc.scalar.activation(
                                    out=pt, in_=st, func=AF.Exp,
                                    bias=miscf_t[:, kc : kc + 1], scale=1.0,
                                )
                            else:
                                nc.vector.tensor_scalar(
                                    out=pt.bitcast(I16), in0=st,
                                    scalar1=SEXP, scalar2=BEXP,
                                    op0=ALU.mult, op1=ALU.add,
                                )
                            pts[(kc, half)] = pt
                        for kind, i in pending.pop(kc, []):
                            _EMIT[kind](i)
                        if deferred and kc in DEFER_AT:
                            deferred[DEFER_AT[kc]]()
                            if DEFER_AT[kc] == 7:
                                deferred = []
                        if kc == 14 and qp < 3:
                            emit_q(qga + 2)
                        if kc == 17 and qp < 3:
                            emit_q(qga + 3)
                        if kc >= lag:
                            for half in range(2):
                                emit_pv(pvt, half * 4, pts.pop((kc - lag, half)), kc - lag)
                    for kc2 in sorted(pending):
                        for kind, i in pending[kc2]:
                            _EMIT[kind](i)
                    # small-nkc safety: fire any deferred closures whose kc
                    # drain point never occurred in this pair's stream
                    for di in sorted(v for k, v in DEFER_AT.items() if k >= nkc):
                        if deferred:
                            deferred[di]()
                    deferred = []
                    for kc in range(nkc - lag, nkc):
                        for half in range(2):
                            emit_pv(pvt, half * 4, pts.pop((kc, half)), kc)
                    deferred = finish_pair(qga, pvt)
                    if qp == 3:
                        for d in deferred:
                            d()
                        deferred = []

    _split_waits(nc)
    return nc


def _numpy_reference(x, block_types, gn_w, gn_b, qkv_w, qkv_b, proj_w, proj_b,
                     is_air, is_wood, is_leaves):
    """Pure-numpy fallback (degenerate case: no non-air keys)."""
    xf = x.reshape(B, C, N).astype(np.float64)
    xs = xf.reshape(B, GROUPS, GSIZE * N)
    mu = xs.mean(axis=2, keepdims=True)
    var = xs.var(axis=2, keepdims=True)
    h = ((xs - mu) / np.sqrt(var + EPS)).reshape(B, C, N)
    h = h * gn_w[None, :, None] + gn_b[None, :, None]
    qkv = np.einsum("oc,bcn->bon", qkv_w.astype(np.float64), h) + qkv_b[None, :, None]
    qkv = qkv.reshape(B, 3, HEADS, HD, N)
    q, k, v = qkv[:, 0], qkv[:, 1], qkv[:, 2]
    attn = np.einsum("bhdn,bhdm->bhnm", q, k) * (HD ** -0.5)
    bf = block_types.reshape(B, N)
    air = is_air[bf]; wood = is_wood[bf]; leaves = is_leaves[bf]
    attn = np.where(air[:, None, None, :] > 0, NEG, attn)
    wo = wood[:, :, None] * wood[:, None, :]
    lo = leaves[:, :, None] * leaves[:, None, :]
    mb = np.clip((wo + lo) * 2.0, 0.0, 10.0)
    pos = np.arange(N); ypos = (pos // W_) % H_
    vm = (np.abs(ypos[None, :] - ypos[:, None]) <= 2).astype(np.float64)
    vb = np.clip(wo * vm[None] * 1.5, 0.0, 10.0)
    attn = attn + (mb + vb)[:, None]
    attn = attn - attn.max(axis=-1, keepdims=True)
    e = np.exp(attn); p = e / e.sum(axis=-1, keepdims=True)
    o = np.einsum("bhnm,bhdm->bhdn", p, v).reshape(B, C, N)
    o = np.einsum("oc,bcn->bon", proj_w.astype(np.float64), o) + proj_b[None, :, None]
    return (xf + o).reshape(x.shape).astype(np.float32)


def kernel(x, block_types, gn_w, gn_b, qkv_w, qkv_b, proj_w, proj_b,
           is_air, is_wood, is_leaves):
    x = np.ascontiguousarray(np.asarray(x, dtype=np.float32))
    gn_w = np.asarray(gn_w, np.float32); gn_b = np.asarray(gn_b, np.float32)
    qkv_w = np.asarray(qkv_w, np.float32); qkv_b = np.asarray(qkv_b, np.float32)
    proj_w = np.asarray(proj_w, np.float32); proj_b = np.asarray(proj_b, np.float32)
    is_air = np.asarray(is_air, np.float32)
    is_wood = np.asarray(is_wood, np.float32)
    is_leaves = np.asarray(is_leaves, np.float32)
    bt = np.asarray(block_types).reshape(N).astype(np.int64)

    x2 = x.reshape(C, N)
    air = is_air[bt]; wood = is_wood[bt]; leaves = is_leaves[bt]
    idx = np.nonzero(air <= 0.0)[0]
    nk = len(idx)
    if nk == 0:
        return _numpy_reference(x, block_types, gn_w, gn_b, qkv_w, qkv_b,
                                proj_w, proj_b, is_air, is_wood, is_leaves)

    nk_pad = ((nk + 127) // 128) * 128
    nkc = nk_pad // 128
    idx_pad = np.concatenate([idx, np.full(nk_pad - nk, idx[0], np.int64)])

    # --- host-side prep: GroupNorm fold + boost features (all O(C*N)) ---
    xs = x2.reshape(GROUPS, GSIZE * N)
    mu = xs.mean(axis=1); var = xs.var(axis=1)
    a_ch = gn_w * np.repeat(1.0 / np.sqrt(var + EPS), GSIZE)
    b_ch = gn_b - np.repeat(mu, GSIZE) * a_ch

    ypos = ((np.arange(N) // W_) % H_).astype(np.int64)
    oneh = np.zeros((N, 16), np.float32); oneh[np.arange(N), ypos] = 1.0
    m16 = (np.abs(np.arange(16)[:, None] - np.arange(16)[None, :]) <= 2).astype(np.float32)
    lfeat = np.ascontiguousarray(np.concatenate(
        [(2.0 * wood)[None], (2.0 * leaves)[None], 1.5 * wood[None] * oneh.T]
    ).astype(NPBF16))                                      # [18, N]
    wood_k = wood[idx_pad]; leaves_k = leaves[idx_pad]
    mk = m16 @ oneh[idx_pad].T
    rfeat = np.ascontiguousarray(np.concatenate(
        [wood_k[None], leaves_k[None], wood_k[None] * mk]
    ).astype(NPBF16))                                      # [18, nk_pad]

    abias = np.zeros(nk_pad, np.float32); abias[nk:] = NEG
    abias = np.ascontiguousarray(abias.reshape(nkc, 128).T)  # [128, nkc]

    identb = np.zeros((128, 161), np.float32)
    identb[:, 0:128] = np.eye(128, dtype=np.float32)
    identb[0:33, 128:161] = np.eye(33, dtype=np.float32)
    identb = identb.astype(NPBF16)

    x2b = np.ascontiguousarray(x2.astype(NPBF16))
    xcb = np.ascontiguousarray(x2[:, idx_pad].astype(NPBF16))

    scale = HD ** -0.5
    shared = {
        "x2b": x2b, "xcb": xcb, "lfeat": lfeat, "rfeat": rfeat, "identb": identb,
        "onesb": np.ones((1, nk_pad), NPBF16),
    }
    in_maps = []
    for hd_i in range(NCORES):
        r0 = hd_i * HD
        wq = qkv_w[r0 : r0 + HD] * scale
        wk = qkv_w[C + r0 : C + r0 + HD]
        wv = qkv_w[2 * C + r0 : 2 * C + r0 + HD]
        cq = wq @ b_ch + qkv_b[r0 : r0 + HD] * scale
        ck = wk @ b_ch + qkv_b[C + r0 : C + r0 + HD]
        wblob = np.zeros((128, 2, 3, HD), np.float32)
        for h in range(2):
            sl = slice(h * 128, (h + 1) * 128)
            wblob[:, h, 0, :] = (wq * a_ch[None, :])[:, sl].T
            wblob[:, h, 1, :] = (wk * a_ch[None, :])[:, sl].T
            wblob[:, h, 2, :] = (wv * a_ch[None, :])[:, sl].T
        miscf = np.zeros((128, nkc + 2), np.float32)
        miscf[:, 0:nkc] = abias
        miscf[0:HD, nkc] = cq
        miscf[0:HD, nkc + 1] = ck
        m = dict(shared)
        m["wblob"] = wblob.astype(NPBF16)
        m["miscf"] = miscf
        m["pwt"] = np.ascontiguousarray(proj_w[:, r0 : r0 + HD].T).astype(NPBF16)
        in_maps.append(m)

    if nk_pad not in _CACHE:
        _CACHE[nk_pad] = _build(nk_pad)
    nc = _CACHE[nk_pad]

    use_trace = TRACE
    if use_trace:
        import importlib.util
        if importlib.util.find_spec("antenv.axon_hooks") is None:
            use_trace = False
    res = run_bass_kernel_spmd(nc, in_maps, core_ids=list(range(NCORES)), trace=use_trace)
    LAST_RESULT["res"] = res

    acc = np.zeros((C, N), np.float32)
    for i in range(NCORES):
        acc += np.asarray(res.results[i]["o"], np.float32)
    # host-folded constants: v-path bias (Wv.b_gn + bv) through projection
    host_c = proj_w @ (qkv_w[2 * C : 3 * C] @ b_ch + qkv_b[2 * C : 3 * C]) + proj_b
    y = x2 + acc + host_c[:, None]
    return y.reshape(B, C, D_, H_, W_).astype(np.float32)
